# revision 14
# baseline (speedup 1.0000x reference)
"""Cost-volume concat kernel for Trainium2 (8 NeuronCores, SPMD).

Problem: left/right (B=4, C=32, H=64, W=128) f32 ->
         out (B, 2C, D=48, H, W) where
  out[b, c,    d, h, w] = left [b, c, h, w]     * (w >= d)
  out[b, C+c,  d, h, w] = right[b, c, h, w - d] * (w >= d)

Sharding: 8 cores = 4 batches x 2 level-halves (d in [24q, 24q+24)); every
core builds both the left and right channel halves for its 24 levels.

The stores go through gpsimd kv_writeback instead of plain DMA.  Work in a
channel-innermost frame: per core and half, y[dd, g, w, cg] with the 2048
(c, h) pairs split into g in [0,16) groups of cg in [0,128).  kv_writeback
writes, per batch entry b and partition p, dho runs of ncn contiguous
elements at per-batch column offset ctx_idx[b]:

  dst[b, p, j, idx[b] : idx[b]+ncn] , src[p, (j*batch_step + b)*ncn + k]

Mapping (ncn = 128, dho = 16, batch = 24 levels, partition p = g*8 + s):
  LEFT : chunk n = s*16 + j + dd at idx[dd] = dd*128; src block t = j + dd;
         partition (g,s) holds blocks P[t] = content[g, w = s*16 + t]
         (zero past w=127).  Writes n < dd are skipped => the masked
         prefix stays at the runtime's pre-zeroed fill; writes n >= 128
         overflow into the next row-group's zero prefix with zero data.
  RIGHT: stored w-REVERSED (host un-reverses): chunk n = s*16 + j at
         idx = 0; P[t] = content[g, w = 127 - s*16 - t] (zero for
         negative index), which makes the per-level source shift t = j+dd
         land on level-independent partition content, and the reversed
         mask zeros come from the same padding.

Level-base 24q is baked host-side: the left content is pre-shifted by 24q
columns, the right content is unshifted; all cores run an identical SPMD
program.  The left half's per-level dst shift idx[dd] = dd*128 is linear
in the batch index, so it is absorbed into batch_stride_bytes (LVL+128
elems per level) and the ctx_idxs are all-zero for both halves -- the idx
tile is a DVE memset, not a load.

Why kv_writeback: each instruction moves batch*2048 512B-runs but the DGE
costs descriptors per 16-partition stripe, so the store phase costs ~16x
less DMA-engine time than HWDGE dma_start (~9us instead of ~114us for the
two halves) and the kernel becomes load-bound (~5MB of SBUF tiles at
~360B/ns).  Each half is 4 writebacks of <=7 levels: one instruction's
~900 ring descriptors must fit the default 1024-entry SWDGE carveout
(the ucode ring size is fixed; enlarging dynamic_dma_scratch_size
crashes on HW).  The ~1.3us desc-gens hide in the gap between each
tile landing and the DMA engines draining the loads.
"""

import sys

for _p in ("/opt/trn_rl_repo",):
    if _p not in sys.path:
        sys.path.append(_p)

import numpy as np

import concourse.bacc as bacc
import concourse.bass as bass
import concourse.mybir as mybir

B, C, H, W = 4, 32, 64, 128
D = 48
NCORES = 8
G, S, DHO, NCN = 16, 8, 16, 128
LCORE = D // 2                 # 24 levels per core
SPAN = DHO + LCORE - 1         # 39 blocks of 128 elems per partition
FREE = SPAN * NCN              # 4992 f32 per partition
LVL = G * 128 * NCN            # 262144 elems per level
NPAD = 8192                    # dram overflow pad (elems)
NCTX = 4096

_F32 = mybir.dt.float32
_I32 = mybir.dt.int32

_NC_CACHE = {}

INSTS = [(0, 7), (7, 7), (14, 7), (21, 3)]   # (level base, batch) per inst


def _build_nc():
    """One SPMD program for every core: 2 HWDGE loads, a memset'd zero
    idx tile, 8 kv_writebacks (4 per half)."""
    nc = bacc.Bacc()
    tl_d = nc.dram_tensor("tl_d", [128, FREE], _F32, kind="ExternalInput")
    tr_d = nc.dram_tensor("tr_d", [128, FREE], _F32, kind="ExternalInput")
    yl = nc.dram_tensor("yl", [LCORE * LVL + NPAD], _F32, kind="ExternalOutput")
    yr = nc.dram_tensor("yr", [LCORE * LVL + NPAD], _F32, kind="ExternalOutput")

    with (
        nc.Block() as block,
        nc.sbuf_tensor("tl", [128, FREE], _F32) as tl,
        nc.sbuf_tensor("tr", [128, FREE], _F32) as tr,
        nc.sbuf_tensor("ix", [128, 8], _I32) as ix,
        nc.semaphore("ld_l") as ld_l,
        nc.semaphore("ld_r") as ld_r,
        nc.semaphore("ixs") as ixs,
        nc.semaphore("wbd") as wbd,
    ):

        @block.sync
        def _(sync):
            sync.dma_start(tl[:, :], tl_d[:, :]).then_inc(ld_l, 16)

        @block.scalar
        def _(scalar):
            scalar.dma_start(tr[:, :], tr_d[:, :]).then_inc(ld_r, 16)

        @block.vector
        def _(vector):
            vector.memset(ix[:, :], 0).then_inc(ixs, 1)

        @block.gpsimd
        def _(gp):
            def wbs(y, t, lvl_stride):
                for l0, L in INSTS:
                    out_ap = bass.AP(
                        y,
                        l0 * lvl_stride,
                        [[lvl_stride, L], [DHO * NCN, 128], [NCN, DHO], [1, NCTX]],
                    )
                    in_ap = bass.AP(
                        t, l0 * NCN, [[FREE, 128], [NCN, DHO], [NCN, L], [1, NCN]]
                    )
                    gp.kv_writeback(out_ap, in_ap, ix[:, :L]).then_inc(wbd, 16)

            gp.wait_ge(ixs, 1)
            gp.wait_ge(ld_l, 16)
            wbs(yl, tl, LVL + NCN)   # left: idx absorbed into batch stride
            gp.wait_ge(ld_r, 16)
            wbs(yr, tr, LVL)         # right: plain level stride
            gp.wait_ge(wbd, 16 * 8)

    nc.compile()
    return nc


def _get_nc():
    if "nc" not in _NC_CACHE:
        _NC_CACHE["nc"] = _build_nc()
    return _NC_CACHE["nc"]


# Partition content gather indices, precomputed once:
#   left : block w-index  M[s, t] = s*16 + t           (invalid -> zero)
#   right: block w-index  M[s, t] = 127 - s*16 - t     (invalid -> zero)
_T = np.arange(SPAN)
_ML = (np.arange(S) * DHO)[:, None] + _T[None, :]            # [S, SPAN]
_MR = 127 - _ML
_VL, _VR = _ML < 128, _MR >= 0


def _content_tiles(half_gw, ml, valid):
    """half_gw: [G, 128(w), 128(cg)] -> SBUF tile [128, FREE]."""
    t = half_gw[:, np.clip(ml, 0, 127), :]                   # [G, S, SPAN, 128]
    t *= valid[None, :, :, None]
    return np.ascontiguousarray(t.reshape(128, FREE))


def _run(left, right, **spmd_kwargs):
    from concourse.bass_utils import run_bass_kernel_spmd

    left = np.ascontiguousarray(np.asarray(left), dtype=np.float32)
    right = np.ascontiguousarray(np.asarray(right), dtype=np.float32)

    in_maps = []
    for k in range(NCORES):
        b, q = divmod(k, 2)
        # [g, w, cg] frames; ch = c*64 + h = g*128 + cg
        lw = left[b].reshape(G, 128, W).transpose(0, 2, 1)
        rw = right[b].reshape(G, 128, W).transpose(0, 2, 1)
        cl = np.zeros((G, 128, 128), np.float32)
        cl[:, : W - 24 * q, :] = lw[:, 24 * q :, :]          # bake level base
        in_maps.append(
            {
                "tl_d": _content_tiles(cl, _ML, _VL),
                "tr_d": _content_tiles(rw, _MR, _VR),
            }
        )

    res = run_bass_kernel_spmd(
        _get_nc(), in_maps, core_ids=list(range(NCORES)), **spmd_kwargs
    )

    out = np.zeros((B, 2 * C, D, H, W), np.float32)
    for k in range(NCORES):
        b, q = divmod(k, 2)
        wlim = W - 24 * q
        dsl = slice(24 * q, 24 * q + LCORE)
        wsl = slice(24 * q, W)

        def stitch(flat, rev):
            y = flat[: LCORE * LVL].reshape(LCORE, G, 128, 128)
            y = y[:, :, 128 - wlim :, :][:, :, ::-1, :] if rev else y[:, :, :wlim, :]
            # [dd, g, wl, cg] -> [c, dd, h, wl]
            y = y.transpose(1, 3, 0, 2).reshape(C, H, LCORE, wlim)
            return y.transpose(0, 2, 1, 3)

        out[b, 0:C, dsl, :, wsl] = stitch(res.results[k]["yl"], False)
        out[b, C:, dsl, :, wsl] = stitch(res.results[k]["yr"], True)
    return out, res


def kernel(left, right):
    out, _ = _run(left, right)
    return out


# revision 22
# speedup vs baseline: 1.1567x; 1.1567x over previous
"""Cost-volume concat kernel for Trainium2 (8 NeuronCores, SPMD).

Problem: left/right (B=4, C=32, H=64, W=128) f32 ->
         out (B, 2C, D=48, H, W) where
  out[b, c,    d, h, w] = left [b, c, h, w]     * (w >= d)
  out[b, C+c,  d, h, w] = right[b, c, h, w - d] * (w >= d)

Sharding: 8 cores = 4 batches x 2 level-halves (d in [24q, 24q+24)); every
core builds both the left and right channel halves for its 24 levels.

The stores go through gpsimd kv_writeback instead of plain DMA.  Work in a
channel-innermost frame: per core and half, y[dd, g, w, cg] with the 2048
(c, h) pairs split into g in [0,16) groups of cg in [0,128).  kv_writeback
writes, per batch entry b and partition p, dho runs of ncn contiguous
elements at per-batch column offset ctx_idx[b]:

  dst[b, p, j, idx[b] : idx[b]+ncn] , src[p, (j*batch_step + b)*ncn + k]

Mapping (ncn = 128, dho = 16, batch = 24 levels, partition p = g*8 + s):
  LEFT : chunk n = s*16 + j + dd at idx[dd] = dd*128; src block t = j + dd;
         partition (g,s) holds blocks P[t] = content[g, w = s*16 + t]
         (zero past w=127).  Writes n < dd are skipped => the masked
         prefix stays at the runtime's pre-zeroed fill; writes n >= 128
         overflow into the next row-group's zero prefix with zero data.
  RIGHT: stored w-REVERSED (host un-reverses): chunk n = s*16 + j at
         idx = 0; P[t] = content[g, w = 127 - s*16 - t] (zero for
         negative index), which makes the per-level source shift t = j+dd
         land on level-independent partition content, and the reversed
         mask zeros come from the same padding.

Level-base 24q is baked host-side: the left content is pre-shifted by 24q
columns, the right content is unshifted; all cores run an identical SPMD
program.  The left half's per-level dst shift idx[dd] = dd*128 is linear
in the batch index, so it is absorbed into batch_stride_bytes (LVL+128
elems per level) and the ctx_idxs are all-zero for both halves -- the idx
tile is a DVE memset, not a load.

Why kv_writeback: each instruction moves batch*2048 512B-runs but the DGE
costs descriptors per 16-partition stripe, so the store phase costs ~16x
less DMA-engine time than HWDGE dma_start (~9us instead of ~114us for the
two halves) and the kernel becomes load-bound.  Each half is 4 writebacks
of <=7 levels: one instruction's ~900 ring descriptors must fit the
default 1024-entry SWDGE carveout (the ucode ring size is fixed;
enlarging dynamic_dma_scratch_size crashes on HW).

The left tile's 39-block span repeats content across partitions
(partition p's blocks [16, 39) are partitions p+1 / p+2's blocks), so
only the unique 16-block prefix is DMA-loaded; the other 23 blocks are
produced on-chip by the otherwise-idle PE as partition-shift matmuls
(f32r, exact x*1+0 copies) through PSUM, drained to SBUF by the ACT
engine.  That takes 1.4MB off the serialized DMA device, and the left
desc-gens start as soon as the chunks they read are expanded.  The right
tile is loaded in full: its expansion would not shorten the critical
path (the Pool desc-gen chain overlaps the right load already).
"""

import sys
from contextlib import ExitStack

for _p in ("/opt/trn_rl_repo",):
    if _p not in sys.path:
        sys.path.append(_p)

import numpy as np

import concourse.bacc as bacc
import concourse.bass as bass
import concourse.mybir as mybir

B, C, H, W = 4, 32, 64, 128
D = 48
NCORES = 8
G, S, DHO, NCN = 16, 8, 16, 128
LCORE = D // 2                 # 24 levels per core
SPAN = DHO + LCORE - 1         # 39 blocks of 128 elems per partition
FREE = SPAN * NCN              # 4992 f32 per partition
LVL = G * 128 * NCN            # 262144 elems per level
NPAD = 8192                    # dram overflow pad (elems)
NCTX = 4096

_F32 = mybir.dt.float32
_F32R = mybir.dt.float32r
_I32 = mybir.dt.int32

_NC_CACHE = {}

INSTS = [(0, 7), (7, 7), (14, 7), (21, 3)]   # (level base, batch) per inst
CBLK = 16 * NCN                # compact (DMA-loaded) prefix: 16 blocks
# expansion chunks: (psum col, sbuf col, ncols, W index)
_CHUNKS = [
    (0, CBLK, 512, 0),               # blocks 16-19   shift-1
    (512, CBLK + 512, 512, 0),       # blocks 20-23
    (1024, CBLK + 1024, 512, 0),     # blocks 24-27
    (1536, CBLK + 1536, 512, 0),     # blocks 28-31
    (2048, 2 * CBLK, 512, 1),        # blocks 32-35   shift-2
    (2560, 2 * CBLK + 512, 384, 1),  # blocks 36-38
]
# exp-sem count each left wb waits for (from the highest block it reads):
#   (0,7)->block 21: chunk 2; (7,7)->28: 4; (14,7)->35: 5; (21,3)->38: 6
_EXP_NEED = [2, 4, 5, 6]


def _build_nc():
    """One SPMD program for every core: 3 HWDGE loads (weights, compact
    left, full right), PE shift-matmul expansion of the left tile, a
    memset'd zero idx tile, 8 kv_writebacks (4 per half)."""
    nc = bacc.Bacc()
    wsh_d = nc.dram_tensor("wsh_d", [128, 256], _F32R, kind="ExternalInput")
    tl_d = nc.dram_tensor("tl_d", [128, CBLK], _F32R, kind="ExternalInput")
    tr_d = nc.dram_tensor("tr_d", [128, FREE], _F32, kind="ExternalInput")
    yl = nc.dram_tensor("yl", [LCORE * LVL + NPAD], _F32R, kind="ExternalOutput")
    yr = nc.dram_tensor("yr", [LCORE * LVL + NPAD], _F32, kind="ExternalOutput")

    with (
        nc.Block() as block,
        nc.sbuf_tensor("tl", [128, FREE], _F32R) as tl,
        nc.sbuf_tensor("tr", [128, FREE], _F32) as tr,
        nc.sbuf_tensor("wt", [128, 256], _F32R) as wt,
        nc.sbuf_tensor("ix", [128, 8], _I32) as ix,
        nc.psum_tensor("ps", [128, 3072], _F32) as ps,
        nc.semaphore("ld_w") as ld_w,
        nc.semaphore("ld_l") as ld_l,
        nc.semaphore("ld_r") as ld_r,
        nc.semaphore("mm") as mm,
        nc.semaphore("exp") as exp,
        nc.semaphore("ixs") as ixs,
        nc.semaphore("wbd") as wbd,
    ):
        # One ring, in order: weights (tiny) -> compact left -> full right,
        # so the left expansion chain starts as early as possible.
        @block.sync
        def _(sync):
            sync.dma_start(wt[:, :], wsh_d[:, :]).then_inc(ld_w, 16)
            sync.dma_start(tl[:, :CBLK], tl_d[:, :]).then_inc(ld_l, 16)
            sync.dma_start(tr[:, :], tr_d[:, :]).then_inc(ld_r, 16)

        @block.vector
        def _(vector):
            vector.memset(ix[:, :], 0).then_inc(ixs, 1)

        @block.tensor
        def _(tensor):
            tensor.wait_ge(ld_w, 16)
            tensor.wait_ge(ld_l, 16)
            for pcol, _scol, n, w in _CHUNKS:
                # shift-1 chunks read source blocks [pcol/128, ...); shift-2
                # chunks (psum cols 2048+) read source blocks [(pcol-2048)/128, ...)
                src0 = pcol - 2048 if w else pcol
                tensor.matmul(
                    ps[:, pcol : pcol + n],
                    wt[:, 128 * w : 128 * (w + 1)],
                    tl[:, src0 : src0 + n],
                    start=True,
                    stop=True,
                ).then_inc(mm, 1)

        @block.scalar
        def _(scalar):
            for k, (pcol, scol, n, _w) in enumerate(_CHUNKS):
                scalar.wait_ge(mm, k + 1)
                scalar.copy(
                    tl[:, scol : scol + n], ps[:, pcol : pcol + n]
                ).then_inc(exp, 1)

        @block.gpsimd
        def _(gp):
            def wbs(y, t, lvl_stride, exp_need):
                for i, (l0, L) in enumerate(INSTS):
                    if exp_need is not None:
                        gp.wait_ge(exp, exp_need[i])
                    out_ap = bass.AP(
                        y,
                        l0 * lvl_stride,
                        [[lvl_stride, L], [DHO * NCN, 128], [NCN, DHO], [1, NCTX]],
                    )
                    in_ap = bass.AP(
                        t, l0 * NCN, [[FREE, 128], [NCN, DHO], [NCN, L], [1, NCN]]
                    )
                    gp.kv_writeback(out_ap, in_ap, ix[:, :L]).then_inc(wbd, 16)

            gp.wait_ge(ixs, 1)
            gp.wait_ge(ld_l, 16)
            wbs(yl, tl, LVL + NCN, _EXP_NEED)  # left: idx folded in stride
            gp.wait_ge(ld_r, 16)
            wbs(yr, tr, LVL, None)             # right: plain level stride
            gp.wait_ge(wbd, 16 * 8)

    nc.compile()
    return nc


def _get_nc():
    if "nc" not in _NC_CACHE:
        _NC_CACHE["nc"] = _build_nc()
    return _NC_CACHE["nc"]


# Partition content gather indices, precomputed once:
#   left : block w-index  M[s, t] = s*16 + t           (invalid -> zero)
#   right: block w-index  M[s, t] = 127 - s*16 - t     (invalid -> zero)
_T = np.arange(SPAN)
_ML = (np.arange(S) * DHO)[:, None] + _T[None, :]            # [S, SPAN]
_MR = 127 - _ML
_VL, _VR = _ML < 128, _MR >= 0


def _content_tiles(half_gw, ml, valid):
    """half_gw: [G, 128(w), 128(cg)] -> SBUF tile [128, FREE]."""
    t = half_gw[:, np.clip(ml, 0, 127), :]                   # [G, S, SPAN, 128]
    t *= valid[None, :, :, None]
    return np.ascontiguousarray(t.reshape(128, FREE))


def _shift_weights():
    """W[k, m] = 1 selects source partition k = m + shift within each
    8-partition s-group (zero columns elsewhere)."""
    w = np.zeros((128, 256), np.float32)
    for m in range(128):
        if m % 8 < 7:
            w[m + 1, m] = 1.0          # W1: shift by one s-slot
        if m % 8 < 6:
            w[m + 2, 128 + m] = 1.0    # W2: shift by two s-slots
    return w


_WSH = _shift_weights()


def _run(left, right, **spmd_kwargs):
    from concourse.bass_utils import run_bass_kernel_spmd

    left = np.ascontiguousarray(np.asarray(left), dtype=np.float32)
    right = np.ascontiguousarray(np.asarray(right), dtype=np.float32)

    in_maps = []
    for k in range(NCORES):
        b, q = divmod(k, 2)
        # [g, w, cg] frames; ch = c*64 + h = g*128 + cg
        lw = left[b].reshape(G, 128, W).transpose(0, 2, 1)
        rw = right[b].reshape(G, 128, W).transpose(0, 2, 1)
        cl = np.zeros((G, 128, 128), np.float32)
        cl[:, : W - 24 * q, :] = lw[:, 24 * q :, :]          # bake level base
        # compact left: partition (g,s) holds only its own 16 blocks
        tl_c = np.ascontiguousarray(cl.reshape(128, CBLK))
        in_maps.append(
            {
                "wsh_d": _WSH,
                "tl_d": tl_c,
                "tr_d": _content_tiles(rw, _MR, _VR),
            }
        )

    res = run_bass_kernel_spmd(
        _get_nc(), in_maps, core_ids=list(range(NCORES)), **spmd_kwargs
    )

    out = np.zeros((B, 2 * C, D, H, W), np.float32)
    for k in range(NCORES):
        b, q = divmod(k, 2)
        wlim = W - 24 * q
        dsl = slice(24 * q, 24 * q + LCORE)
        wsl = slice(24 * q, W)

        def stitch(flat, rev):
            y = flat[: LCORE * LVL].reshape(LCORE, G, 128, 128)
            y = y[:, :, 128 - wlim :, :][:, :, ::-1, :] if rev else y[:, :, :wlim, :]
            # [dd, g, wl, cg] -> [c, dd, h, wl]
            y = y.transpose(1, 3, 0, 2).reshape(C, H, LCORE, wlim)
            return y.transpose(0, 2, 1, 3)

        out[b, 0:C, dsl, :, wsl] = stitch(res.results[k]["yl"], False)
        out[b, C:, dsl, :, wsl] = stitch(res.results[k]["yr"], True)
    return out, res


def kernel(left, right):
    out, _ = _run(left, right)
    return out


# revision 24
# speedup vs baseline: 1.1977x; 1.0354x over previous
"""Cost-volume concat kernel for Trainium2 (8 NeuronCores, SPMD).

Problem: left/right (B=4, C=32, H=64, W=128) f32 ->
         out (B, 2C, D=48, H, W) where
  out[b, c,    d, h, w] = left [b, c, h, w]     * (w >= d)
  out[b, C+c,  d, h, w] = right[b, c, h, w - d] * (w >= d)

Sharding: 8 cores = 4 batches x 2 level-halves (d in [24q, 24q+24)); every
core builds both the left and right channel halves for its 24 levels.

The stores go through gpsimd kv_writeback instead of plain DMA.  Work in a
channel-innermost frame: per core and half, y[dd, g, w, cg] with the 2048
(c, h) pairs split into g in [0,16) groups of cg in [0,128).  kv_writeback
writes, per batch entry b and partition p, dho runs of ncn contiguous
elements at per-batch column offset ctx_idx[b]:

  dst[b, p, j, idx[b] : idx[b]+ncn] , src[p, (j*batch_step + b)*ncn + k]

Mapping (ncn = 128, dho = 16, batch = 24 levels, partition p = g*8 + s):
  LEFT : chunk n = s*16 + j + dd at idx[dd] = dd*128; src block t = j + dd;
         partition (g,s) holds blocks P[t] = content[g, w = s*16 + t]
         (zero past w=127).  Writes n < dd are skipped => the masked
         prefix stays at the runtime's pre-zeroed fill; writes n >= 128
         overflow into the next row-group's zero prefix with zero data.
  RIGHT: stored w-REVERSED (host un-reverses): chunk n = s*16 + j at
         idx = 0; P[t] = content[g, w = 127 - s*16 - t] (zero for
         negative index), which makes the per-level source shift t = j+dd
         land on level-independent partition content, and the reversed
         mask zeros come from the same padding.

Level-base 24q is baked host-side: the left content is pre-shifted by 24q
columns, the right content is unshifted; all cores run an identical SPMD
program.  The left half's per-level dst shift idx[dd] = dd*128 is linear
in the batch index, so it is absorbed into batch_stride_bytes (LVL+128
elems per level) and the ctx_idxs are all-zero for both halves -- the idx
tile is a DVE memset, not a load.

Why kv_writeback: each instruction moves batch*2048 512B-runs but the DGE
costs descriptors per 16-partition stripe, so the store phase costs ~16x
less DMA-engine time than HWDGE dma_start (~9us instead of ~114us for the
two halves) and the kernel becomes load-bound.  Each half is 4 writebacks
of <=7 levels: one instruction's ~900 ring descriptors must fit the
default 1024-entry SWDGE carveout (the ucode ring size is fixed;
enlarging dynamic_dma_scratch_size crashes on HW).

The left tile's 39-block span repeats content across partitions
(partition p's blocks [16, 39) are partitions p+1 / p+2's blocks), so
only the unique 16-block prefix is DMA-loaded; the other 23 blocks are
produced on-chip by the otherwise-idle PE as partition-shift matmuls
(f32r, exact x*1+0 copies) through PSUM, drained to SBUF by the ACT
engine.  That takes 1.4MB off the serialized DMA device, and the left
desc-gens start as soon as the chunks they read are expanded.  The right
tile is loaded in full: its expansion would not shorten the critical
path (the Pool desc-gen chain overlaps the right load already).
"""

import sys
from contextlib import ExitStack

for _p in ("/opt/trn_rl_repo",):
    if _p not in sys.path:
        sys.path.append(_p)

import numpy as np

import concourse.bacc as bacc
import concourse.bass as bass
import concourse.mybir as mybir

B, C, H, W = 4, 32, 64, 128
D = 48
NCORES = 8
G, S, DHO, NCN = 16, 8, 16, 128
LCORE = D // 2                 # 24 levels per core
SPAN = DHO + LCORE - 1         # 39 blocks of 128 elems per partition
FREE = SPAN * NCN              # 4992 f32 per partition
LVL = G * 128 * NCN            # 262144 elems per level
NPAD = 8192                    # dram overflow pad (elems)
NCTX = 4096

_F32 = mybir.dt.float32
_F32R = mybir.dt.float32r
_I32 = mybir.dt.int32

_NC_CACHE = {}

# (level base, batch) per writeback; the left split leads with a
# compact-only instruction so its desc-gen starts before any expansion.
INSTS_L = [(0, 1), (1, 6), (7, 7), (14, 7), (21, 3)]
INSTS_R = [(0, 7), (7, 7), (14, 7), (21, 3)]
CBLK = 16 * NCN                # compact (DMA-loaded) prefix: 16 blocks
# expansion chunks: (psum col, sbuf col, ncols, W index)
_CHUNKS = [
    (0, CBLK, 512, 0),               # blocks 16-19   shift-1
    (512, CBLK + 512, 512, 0),       # blocks 20-23
    (1024, CBLK + 1024, 512, 0),     # blocks 24-27
    (1536, CBLK + 1536, 512, 0),     # blocks 28-31
    (2048, 2 * CBLK, 512, 1),        # blocks 32-35   shift-2
    (2560, 2 * CBLK + 512, 384, 1),  # blocks 36-38
]
# exp-sem count each wb waits for (from the highest block it reads);
# 0 = compact prefix only, no expansion dependency.
_EXP_NEED_L = [0, 2, 4, 5, 6]
_EXP_NEED_R = [2, 4, 5, 6]


def _build_nc():
    """One SPMD program for every core: 3 HWDGE loads (weights, compact
    left, full right), PE shift-matmul expansion of the left tile, a
    memset'd zero idx tile, 8 kv_writebacks (4 per half)."""
    nc = bacc.Bacc()
    wsh_d = nc.dram_tensor("wsh_d", [128, 256], _F32R, kind="ExternalInput")
    tl_d = nc.dram_tensor("tl_d", [128, CBLK], _F32R, kind="ExternalInput")
    tr_d = nc.dram_tensor("tr_d", [128, CBLK], _F32R, kind="ExternalInput")
    yl = nc.dram_tensor("yl", [LCORE * LVL + NPAD], _F32R, kind="ExternalOutput")
    yr = nc.dram_tensor("yr", [LCORE * LVL + NPAD], _F32R, kind="ExternalOutput")

    with (
        nc.Block() as block,
        nc.sbuf_tensor("tl", [128, FREE], _F32R) as tl,
        nc.sbuf_tensor("tr", [128, FREE], _F32R) as tr,
        nc.sbuf_tensor("wt", [128, 256], _F32R) as wt,
        nc.sbuf_tensor("ix", [128, 8], _I32) as ix,
        nc.psum_tensor("ps", [128, 3072], _F32) as ps,
        nc.psum_tensor("ps2", [128, 1024], _F32) as ps2,
        nc.semaphore("ld_w") as ld_w,
        nc.semaphore("ld_l") as ld_l,
        nc.semaphore("ld_r") as ld_r,
        nc.semaphore("mm") as mm,
        nc.semaphore("mm_r") as mm_r,
        nc.semaphore("exp") as exp,
        nc.semaphore("exp_r") as exp_r,
        nc.semaphore("ixs") as ixs,
        nc.semaphore("wbd") as wbd,
    ):
        # One ring, in order: weights (tiny) -> compact left -> full right,
        # so the left expansion chain starts as early as possible.
        @block.sync
        def _(sync):
            sync.dma_start(wt[:, :], wsh_d[:, :]).then_inc(ld_w, 16)
            sync.dma_start(tl[:, :CBLK], tl_d[:, :]).then_inc(ld_l, 16)
            sync.dma_start(tr[:, :CBLK], tr_d[:, :]).then_inc(ld_r, 16)

        @block.vector
        def _(vector):
            vector.memset(ix[:, :], 0).then_inc(ixs, 1)
            for c, (_pcol, scol, n, _w) in enumerate(_CHUNKS):
                vector.wait_ge(mm_r, c + 1)
                vector.tensor_copy(
                    tr[:, scol : scol + n], ps2[:, (c % 2) * 512 : (c % 2) * 512 + n]
                ).then_inc(exp_r, 1)

        @block.tensor
        def _(tensor):
            tensor.wait_ge(ld_w, 16)
            tensor.wait_ge(ld_l, 16)
            for pcol, _scol, n, w in _CHUNKS:
                # shift-1 chunks read source blocks [pcol/128, ...); shift-2
                # chunks (psum cols 2048+) read source blocks [(pcol-2048)/128, ...)
                src0 = pcol - 2048 if w else pcol
                tensor.matmul(
                    ps[:, pcol : pcol + n],
                    wt[:, 128 * w : 128 * (w + 1)],
                    tl[:, src0 : src0 + n],
                    start=True,
                    stop=True,
                ).then_inc(mm, 1)
            # right half: rotate through ps2's two banks; wait for the DVE
            # drain of the bank before reusing it
            tensor.wait_ge(ld_r, 16)
            for c, (pcol, _scol, n, w) in enumerate(_CHUNKS):
                if c >= 2:
                    tensor.wait_ge(exp_r, c - 1)
                src0 = pcol - 2048 if w else pcol
                tensor.matmul(
                    ps2[:, (c % 2) * 512 : (c % 2) * 512 + n],
                    wt[:, 128 * w : 128 * (w + 1)],
                    tr[:, src0 : src0 + n],
                    start=True,
                    stop=True,
                ).then_inc(mm_r, 1)

        @block.scalar
        def _(scalar):
            for k, (pcol, scol, n, _w) in enumerate(_CHUNKS):
                scalar.wait_ge(mm, k + 1)
                scalar.copy(
                    tl[:, scol : scol + n], ps[:, pcol : pcol + n]
                ).then_inc(exp, 1)

        @block.gpsimd
        def _(gp):
            def wbs(y, t, lvl_stride, insts, exp_sem, exp_need):
                for i, (l0, L) in enumerate(insts):
                    if exp_need[i]:
                        gp.wait_ge(exp_sem, exp_need[i])
                    out_ap = bass.AP(
                        y,
                        l0 * lvl_stride,
                        [[lvl_stride, L], [DHO * NCN, 128], [NCN, DHO], [1, NCTX]],
                    )
                    in_ap = bass.AP(
                        t, l0 * NCN, [[FREE, 128], [NCN, DHO], [NCN, L], [1, NCN]]
                    )
                    gp.kv_writeback(out_ap, in_ap, ix[:, :L]).then_inc(wbd, 16)

            gp.wait_ge(ixs, 1)
            gp.wait_ge(ld_l, 16)
            wbs(yl, tl, LVL + NCN, INSTS_L, exp, _EXP_NEED_L)
            gp.wait_ge(ld_r, 16)
            wbs(yr, tr, LVL, INSTS_R, exp_r, _EXP_NEED_R)
            gp.wait_ge(wbd, 16 * 9)

    nc.compile()
    return nc


def _get_nc():
    if "nc" not in _NC_CACHE:
        _NC_CACHE["nc"] = _build_nc()
    return _NC_CACHE["nc"]


# Partition content gather indices, precomputed once:
#   left : block w-index  M[s, t] = s*16 + t           (invalid -> zero)
#   right: block w-index  M[s, t] = 127 - s*16 - t     (invalid -> zero)
# right compact: partition (g,s) block u holds reversed content
# cr[g, 127 - (s*16 + u)]; all indices valid for u < 16.
_MR16 = 127 - ((np.arange(S) * DHO)[:, None] + np.arange(16)[None, :])


def _shift_weights():
    """W[k, m] = 1 selects source partition k = m + shift within each
    8-partition s-group (zero columns elsewhere)."""
    w = np.zeros((128, 256), np.float32)
    for m in range(128):
        if m % 8 < 7:
            w[m + 1, m] = 1.0          # W1: shift by one s-slot
        if m % 8 < 6:
            w[m + 2, 128 + m] = 1.0    # W2: shift by two s-slots
    return w


_WSH = _shift_weights()


def _run(left, right, **spmd_kwargs):
    from concourse.bass_utils import run_bass_kernel_spmd

    left = np.ascontiguousarray(np.asarray(left), dtype=np.float32)
    right = np.ascontiguousarray(np.asarray(right), dtype=np.float32)

    in_maps = []
    for k in range(NCORES):
        b, q = divmod(k, 2)
        # [g, w, cg] frames; ch = c*64 + h = g*128 + cg
        lw = left[b].reshape(G, 128, W).transpose(0, 2, 1)
        rw = right[b].reshape(G, 128, W).transpose(0, 2, 1)
        cl = np.zeros((G, 128, 128), np.float32)
        cl[:, : W - 24 * q, :] = lw[:, 24 * q :, :]          # bake level base
        # compact tiles: each partition (g,s) holds only its own 16 blocks
        tl_c = np.ascontiguousarray(cl.reshape(128, CBLK))
        tr_c = np.ascontiguousarray(rw[:, _MR16, :].reshape(128, CBLK))
        in_maps.append({"wsh_d": _WSH, "tl_d": tl_c, "tr_d": tr_c})

    res = run_bass_kernel_spmd(
        _get_nc(), in_maps, core_ids=list(range(NCORES)), **spmd_kwargs
    )

    out = np.zeros((B, 2 * C, D, H, W), np.float32)
    for k in range(NCORES):
        b, q = divmod(k, 2)
        wlim = W - 24 * q
        dsl = slice(24 * q, 24 * q + LCORE)
        wsl = slice(24 * q, W)

        def stitch(flat, rev):
            y = flat[: LCORE * LVL].reshape(LCORE, G, 128, 128)
            y = y[:, :, 128 - wlim :, :][:, :, ::-1, :] if rev else y[:, :, :wlim, :]
            # [dd, g, wl, cg] -> [c, dd, h, wl]
            y = y.transpose(1, 3, 0, 2).reshape(C, H, LCORE, wlim)
            return y.transpose(0, 2, 1, 3)

        out[b, 0:C, dsl, :, wsl] = stitch(res.results[k]["yl"], False)
        out[b, C:, dsl, :, wsl] = stitch(res.results[k]["yr"], True)
    return out, res


def kernel(left, right):
    out, _ = _run(left, right)
    return out


# revision 26
# speedup vs baseline: 1.2142x; 1.0138x over previous
"""Cost-volume concat kernel for Trainium2 (8 NeuronCores, SPMD).

Problem: left/right (B=4, C=32, H=64, W=128) f32 ->
         out (B, 2C, D=48, H, W) where
  out[b, c,    d, h, w] = left [b, c, h, w]     * (w >= d)
  out[b, C+c,  d, h, w] = right[b, c, h, w - d] * (w >= d)

Sharding: 8 cores = 4 batches x 2 level-halves (d in [24q, 24q+24)); every
core builds both the left and right channel halves for its 24 levels.

The stores go through gpsimd kv_writeback instead of plain DMA.  Work in a
channel-innermost frame: per core and half, y[dd, g, w, cg] with the 2048
(c, h) pairs split into g in [0,16) groups of cg in [0,128).  kv_writeback
writes, per batch entry b and partition p, dho runs of ncn contiguous
elements at per-batch column offset ctx_idx[b]:

  dst[b, p, j, idx[b] : idx[b]+ncn] , src[p, (j*batch_step + b)*ncn + k]

Mapping (ncn = 128, dho = 16, batch = 24 levels, partition p = g*8 + s):
  LEFT : chunk n = s*16 + j + dd at idx[dd] = dd*128; src block t = j + dd;
         partition (g,s) holds blocks P[t] = content[g, w = s*16 + t]
         (zero past w=127).  Writes n < dd are skipped => the masked
         prefix stays at the runtime's pre-zeroed fill; writes n >= 128
         overflow into the next row-group's zero prefix with zero data.
  RIGHT: stored w-REVERSED (host un-reverses): chunk n = s*16 + j at
         idx = 0; P[t] = content[g, w = 127 - s*16 - t] (zero for
         negative index), which makes the per-level source shift t = j+dd
         land on level-independent partition content, and the reversed
         mask zeros come from the same padding.

Level-base 24q is baked host-side: the left content is pre-shifted by 24q
columns, the right content is unshifted; all cores run an identical SPMD
program.  The left half's per-level dst shift idx[dd] = dd*128 is linear
in the batch index, so it is absorbed into batch_stride_bytes (LVL+128
elems per level) and the ctx_idxs are all-zero for both halves -- the idx
tile is a DVE memset, not a load.

Why kv_writeback: each instruction moves batch*2048 512B-runs but the DGE
costs descriptors per 16-partition stripe, so the store phase costs ~16x
less DMA-engine time than HWDGE dma_start (~9us instead of ~114us for the
two halves) and the kernel becomes load-bound.  Each half is 4 writebacks
of <=7 levels: one instruction's ~900 ring descriptors must fit the
default 1024-entry SWDGE carveout (the ucode ring size is fixed;
enlarging dynamic_dma_scratch_size crashes on HW).

The left tile's 39-block span repeats content across partitions
(partition p's blocks [16, 39) are partitions p+1 / p+2's blocks), so
only the unique 16-block prefix is DMA-loaded; the other 23 blocks are
produced on-chip by the otherwise-idle PE as partition-shift matmuls
(f32r, exact x*1+0 copies) through PSUM, drained to SBUF by the ACT
engine.  That takes 1.4MB off the serialized DMA device, and the left
desc-gens start as soon as the chunks they read are expanded.  The right
tile is loaded in full: its expansion would not shorten the critical
path (the Pool desc-gen chain overlaps the right load already).
"""

import sys
from contextlib import ExitStack

for _p in ("/opt/trn_rl_repo",):
    if _p not in sys.path:
        sys.path.append(_p)

import numpy as np

import concourse.bacc as bacc
import concourse.bass as bass
import concourse.mybir as mybir

B, C, H, W = 4, 32, 64, 128
D = 48
NCORES = 8
G, S, DHO, NCN = 16, 8, 16, 128
LCORE = D // 2                 # 24 levels per core
SPAN = DHO + LCORE - 1         # 39 blocks of 128 elems per partition
FREE = SPAN * NCN              # 4992 f32 per partition
LVL = G * 128 * NCN            # 262144 elems per level
NPAD = 8192                    # dram overflow pad (elems)
NCTX = 4096

_F32 = mybir.dt.float32
_F32R = mybir.dt.float32r
_I32 = mybir.dt.int32

_NC_CACHE = {}

# (level base, batch) per writeback; the left split leads with a
# compact-only instruction so its desc-gen starts before any expansion.
INSTS_L = [(0, 1), (1, 6), (7, 7), (14, 7), (21, 3)]
INSTS_R = [(0, 7), (7, 7), (14, 7), (21, 3)]
CBLK = 16 * NCN                # compact (DMA-loaded) prefix: 16 blocks
# expansion chunks: (psum col, sbuf col, ncols, W index)
_CHUNKS = [
    (0, CBLK, 512, 0),               # blocks 16-19   shift-1
    (512, CBLK + 512, 512, 0),       # blocks 20-23
    (1024, CBLK + 1024, 512, 0),     # blocks 24-27
    (1536, CBLK + 1536, 512, 0),     # blocks 28-31
    (2048, 2 * CBLK, 512, 1),        # blocks 32-35   shift-2
    (2560, 2 * CBLK + 512, 384, 1),  # blocks 36-38
]
# exp-sem count each wb waits for (from the highest block it reads);
# 0 = compact prefix only, no expansion dependency.
_EXP_NEED_L = [0, 2, 4, 5, 6]
# right chunks drain on two engines: DVE owns bank0/chunks 0,2,4 (exp_ra),
# ACT owns bank1/chunks 1,3,5 (exp_rb); per-wb (ra, rb) minimums:
_EXP_NEED_R = [(1, 1), (2, 2), (3, 2), (3, 3)]


def _build_nc():
    """One SPMD program for every core: 3 HWDGE loads (weights, compact
    left, full right), PE shift-matmul expansion of the left tile, a
    memset'd zero idx tile, 8 kv_writebacks (4 per half)."""
    nc = bacc.Bacc()
    wsh_d = nc.dram_tensor("wsh_d", [128, 256], _F32R, kind="ExternalInput")
    tl_d = nc.dram_tensor("tl_d", [128, CBLK], _F32R, kind="ExternalInput")
    tr_d = nc.dram_tensor("tr_d", [128, CBLK], _F32R, kind="ExternalInput")
    yl = nc.dram_tensor("yl", [LCORE * LVL + NPAD], _F32R, kind="ExternalOutput")
    yr = nc.dram_tensor("yr", [LCORE * LVL + NPAD], _F32R, kind="ExternalOutput")

    with (
        nc.Block() as block,
        nc.sbuf_tensor("tl", [128, FREE], _F32R) as tl,
        nc.sbuf_tensor("tr", [128, FREE], _F32R) as tr,
        nc.sbuf_tensor("wt", [128, 256], _F32R) as wt,
        nc.sbuf_tensor("ix", [128, 8], _I32) as ix,
        nc.psum_tensor("ps", [128, 3072], _F32) as ps,
        nc.psum_tensor("ps2", [128, 1024], _F32) as ps2,
        nc.semaphore("ld_w") as ld_w,
        nc.semaphore("ld_l") as ld_l,
        nc.semaphore("ld_r") as ld_r,
        nc.semaphore("mm") as mm,
        nc.semaphore("mm_r") as mm_r,
        nc.semaphore("exp") as exp,
        nc.semaphore("exp_ra") as exp_ra,
        nc.semaphore("exp_rb") as exp_rb,
        nc.semaphore("ixs") as ixs,
        nc.semaphore("wbd") as wbd,
    ):
        # One ring, in order: weights (tiny) -> compact left -> full right,
        # so the left expansion chain starts as early as possible.
        @block.sync
        def _(sync):
            sync.dma_start(tl[:, :CBLK], tl_d[:, :]).then_inc(ld_l, 16)
            sync.dma_start(tr[:, :CBLK], tr_d[:, :]).then_inc(ld_r, 16)

        @block.scalar
        def _(scalar):
            scalar.dma_start(wt[:, :], wsh_d[:, :]).then_inc(ld_w, 16)

        @block.vector
        def _(vector):
            vector.memset(ix[:, :], 0).then_inc(ixs, 1)
            for c in (0, 2, 4):
                _pcol, scol, n, _w = _CHUNKS[c]
                vector.wait_ge(mm_r, c + 1)
                vector.tensor_copy(
                    tr[:, scol : scol + n], ps2[:, :n]
                ).then_inc(exp_ra, 1)

        @block.tensor
        def _(tensor):
            tensor.wait_ge(ld_w, 16)
            tensor.wait_ge(ld_l, 16)
            for pcol, _scol, n, w in _CHUNKS:
                # shift-1 chunks read source blocks [pcol/128, ...); shift-2
                # chunks (psum cols 2048+) read source blocks [(pcol-2048)/128, ...)
                src0 = pcol - 2048 if w else pcol
                tensor.matmul(
                    ps[:, pcol : pcol + n],
                    wt[:, 128 * w : 128 * (w + 1)],
                    tl[:, src0 : src0 + n],
                    start=True,
                    stop=True,
                ).then_inc(mm, 1)
            # right half: rotate through ps2's two banks; each bank is
            # drained by its own engine (DVE bank0, ACT bank1)
            tensor.wait_ge(ld_r, 16)
            for c, (pcol, _scol, n, w) in enumerate(_CHUNKS):
                if c >= 2:
                    tensor.wait_ge(exp_ra if c % 2 == 0 else exp_rb, c // 2)
                src0 = pcol - 2048 if w else pcol
                tensor.matmul(
                    ps2[:, (c % 2) * 512 : (c % 2) * 512 + n],
                    wt[:, 128 * w : 128 * (w + 1)],
                    tr[:, src0 : src0 + n],
                    start=True,
                    stop=True,
                ).then_inc(mm_r, 1)

        @block.scalar
        def _(scalar):
            for k, (pcol, scol, n, _w) in enumerate(_CHUNKS):
                scalar.wait_ge(mm, k + 1)
                scalar.copy(
                    tl[:, scol : scol + n], ps[:, pcol : pcol + n]
                ).then_inc(exp, 1)
            for c in (1, 3, 5):
                _pcol, scol, n, _w = _CHUNKS[c]
                scalar.wait_ge(mm_r, c + 1)
                scalar.copy(
                    tr[:, scol : scol + n], ps2[:, 512 : 512 + n]
                ).then_inc(exp_rb, 1)

        @block.gpsimd
        def _(gp):
            def wbs(y, t, lvl_stride, insts, exp_sem, exp_need):
                for i, (l0, L) in enumerate(insts):
                    if exp_need[i]:
                        gp.wait_ge(exp_sem, exp_need[i])
                    out_ap = bass.AP(
                        y,
                        l0 * lvl_stride,
                        [[lvl_stride, L], [DHO * NCN, 128], [NCN, DHO], [1, NCTX]],
                    )
                    in_ap = bass.AP(
                        t, l0 * NCN, [[FREE, 128], [NCN, DHO], [NCN, L], [1, NCN]]
                    )
                    gp.kv_writeback(out_ap, in_ap, ix[:, :L]).then_inc(wbd, 16)

            gp.wait_ge(ixs, 1)
            gp.wait_ge(ld_l, 16)
            wbs(yl, tl, LVL + NCN, INSTS_L, exp, _EXP_NEED_L)
            gp.wait_ge(ld_r, 16)
            for i, (l0, L) in enumerate(INSTS_R):
                ra, rb = _EXP_NEED_R[i]
                gp.wait_ge(exp_ra, ra)
                gp.wait_ge(exp_rb, rb)
                out_ap = bass.AP(
                    yr, l0 * LVL,
                    [[LVL, L], [DHO * NCN, 128], [NCN, DHO], [1, NCTX]],
                )
                in_ap = bass.AP(
                    tr, l0 * NCN, [[FREE, 128], [NCN, DHO], [NCN, L], [1, NCN]]
                )
                gp.kv_writeback(out_ap, in_ap, ix[:, :L]).then_inc(wbd, 16)
            gp.wait_ge(wbd, 16 * 9)

    nc.compile()
    return nc


def _get_nc():
    if "nc" not in _NC_CACHE:
        _NC_CACHE["nc"] = _build_nc()
    return _NC_CACHE["nc"]


# Partition content gather indices, precomputed once:
#   left : block w-index  M[s, t] = s*16 + t           (invalid -> zero)
#   right: block w-index  M[s, t] = 127 - s*16 - t     (invalid -> zero)
# right compact: partition (g,s) block u holds reversed content
# cr[g, 127 - (s*16 + u)]; all indices valid for u < 16.
_MR16 = 127 - ((np.arange(S) * DHO)[:, None] + np.arange(16)[None, :])


def _shift_weights():
    """W[k, m] = 1 selects source partition k = m + shift within each
    8-partition s-group (zero columns elsewhere)."""
    w = np.zeros((128, 256), np.float32)
    for m in range(128):
        if m % 8 < 7:
            w[m + 1, m] = 1.0          # W1: shift by one s-slot
        if m % 8 < 6:
            w[m + 2, 128 + m] = 1.0    # W2: shift by two s-slots
    return w


_WSH = _shift_weights()


def _run(left, right, **spmd_kwargs):
    from concourse.bass_utils import run_bass_kernel_spmd

    left = np.ascontiguousarray(np.asarray(left), dtype=np.float32)
    right = np.ascontiguousarray(np.asarray(right), dtype=np.float32)

    in_maps = []
    for k in range(NCORES):
        b, q = divmod(k, 2)
        # [g, w, cg] frames; ch = c*64 + h = g*128 + cg
        lw = left[b].reshape(G, 128, W).transpose(0, 2, 1)
        rw = right[b].reshape(G, 128, W).transpose(0, 2, 1)
        cl = np.zeros((G, 128, 128), np.float32)
        cl[:, : W - 24 * q, :] = lw[:, 24 * q :, :]          # bake level base
        # compact tiles: each partition (g,s) holds only its own 16 blocks
        tl_c = np.ascontiguousarray(cl.reshape(128, CBLK))
        tr_c = np.ascontiguousarray(rw[:, _MR16, :].reshape(128, CBLK))
        in_maps.append({"wsh_d": _WSH, "tl_d": tl_c, "tr_d": tr_c})

    res = run_bass_kernel_spmd(
        _get_nc(), in_maps, core_ids=list(range(NCORES)), **spmd_kwargs
    )

    out = np.zeros((B, 2 * C, D, H, W), np.float32)
    for k in range(NCORES):
        b, q = divmod(k, 2)
        wlim = W - 24 * q
        dsl = slice(24 * q, 24 * q + LCORE)
        wsl = slice(24 * q, W)

        def stitch(flat, rev):
            y = flat[: LCORE * LVL].reshape(LCORE, G, 128, 128)
            y = y[:, :, 128 - wlim :, :][:, :, ::-1, :] if rev else y[:, :, :wlim, :]
            # [dd, g, wl, cg] -> [c, dd, h, wl]
            y = y.transpose(1, 3, 0, 2).reshape(C, H, LCORE, wlim)
            return y.transpose(0, 2, 1, 3)

        out[b, 0:C, dsl, :, wsl] = stitch(res.results[k]["yl"], False)
        out[b, C:, dsl, :, wsl] = stitch(res.results[k]["yr"], True)
    return out, res


def kernel(left, right):
    out, _ = _run(left, right)
    return out


# revision 27
# speedup vs baseline: 1.2600x; 1.0377x over previous
"""Cost-volume concat kernel for Trainium2 (8 NeuronCores, SPMD).

Problem: left/right (B=4, C=32, H=64, W=128) f32 ->
         out (B, 2C, D=48, H, W) where
  out[b, c,    d, h, w] = left [b, c, h, w]     * (w >= d)
  out[b, C+c,  d, h, w] = right[b, c, h, w - d] * (w >= d)

Sharding: 8 cores = 4 batches x 2 level-halves (d in [24q, 24q+24)); every
core builds both the left and right channel halves for its 24 levels.

The stores go through gpsimd kv_writeback instead of plain DMA.  Work in a
channel-innermost frame: per core and half, y[dd, g, w, cg] with the 2048
(c, h) pairs split into g in [0,16) groups of cg in [0,128).  kv_writeback
writes, per batch entry b and partition p, dho runs of ncn contiguous
elements at per-batch column offset ctx_idx[b]:

  dst[b, p, j, idx[b] : idx[b]+ncn] , src[p, (j*batch_step + b)*ncn + k]

Mapping (ncn = 128, dho = 16, batch = 24 levels, partition p = g*8 + s):
  LEFT : chunk n = s*16 + j + dd at idx[dd] = dd*128; src block t = j + dd;
         partition (g,s) holds blocks P[t] = content[g, w = s*16 + t]
         (zero past w=127).  Writes n < dd are skipped => the masked
         prefix stays at the runtime's pre-zeroed fill; writes n >= 128
         overflow into the next row-group's zero prefix with zero data.
  RIGHT: stored w-REVERSED (host un-reverses): chunk n = s*16 + j at
         idx = 0; P[t] = content[g, w = 127 - s*16 - t] (zero for
         negative index), which makes the per-level source shift t = j+dd
         land on level-independent partition content, and the reversed
         mask zeros come from the same padding.

Level-base 24q is baked host-side: the left content is pre-shifted by 24q
columns, the right content is unshifted; all cores run an identical SPMD
program.  The left half's per-level dst shift idx[dd] = dd*128 is linear
in the batch index, so it is absorbed into batch_stride_bytes (LVL+128
elems per level) and the ctx_idxs are all-zero for both halves -- the idx
tile is a DVE memset, not a load.

Why kv_writeback: each instruction moves batch*2048 512B-runs but the DGE
costs descriptors per 16-partition stripe, so the store phase costs ~16x
less DMA-engine time than HWDGE dma_start (~9us instead of ~114us for the
two halves) and the kernel becomes load-bound.  Each half is 4 writebacks
of <=7 levels: one instruction's ~900 ring descriptors must fit the
default 1024-entry SWDGE carveout (the ucode ring size is fixed;
enlarging dynamic_dma_scratch_size crashes on HW).

The left tile's 39-block span repeats content across partitions
(partition p's blocks [16, 39) are partitions p+1 / p+2's blocks), so
only the unique 16-block prefix is DMA-loaded; the other 23 blocks are
produced on-chip by the otherwise-idle PE as partition-shift matmuls
(f32r, exact x*1+0 copies) through PSUM, drained to SBUF by the ACT
engine.  That takes 1.4MB off the serialized DMA device, and the left
desc-gens start as soon as the chunks they read are expanded.  The right
tile is loaded in full: its expansion would not shorten the critical
path (the Pool desc-gen chain overlaps the right load already).
"""

import sys
from contextlib import ExitStack

for _p in ("/opt/trn_rl_repo",):
    if _p not in sys.path:
        sys.path.append(_p)

import numpy as np

import concourse.bacc as bacc
import concourse.bass as bass
import concourse.mybir as mybir

B, C, H, W = 4, 32, 64, 128
D = 48
NCORES = 8
G, S, DHO, NCN = 16, 8, 16, 128
LCORE = D // 2                 # 24 levels per core
SPAN = DHO + LCORE - 1         # 39 blocks of 128 elems per partition
FREE = SPAN * NCN              # 4992 f32 per partition
LVL = G * 128 * NCN            # 262144 elems per level
NPAD = 8192                    # dram overflow pad (elems)
NCTX = 4096

_F32 = mybir.dt.float32
_F32R = mybir.dt.float32r
_I32 = mybir.dt.int32

_NC_CACHE = {}

# (level base, batch) per writeback; the left split leads with a
# compact-only instruction so its desc-gen starts before any expansion.
INSTS_L = [(0, 1), (1, 6), (7, 7), (14, 7), (21, 3)]
INSTS_R = [(0, 7), (7, 7), (14, 7), (21, 3)]
CBLK = 16 * NCN                # compact (DMA-loaded) prefix: 16 blocks
# expansion chunks: (psum col, sbuf col, ncols, W index)
_CHUNKS = [
    (0, CBLK, 512, 0),               # blocks 16-19   shift-1
    (512, CBLK + 512, 512, 0),       # blocks 20-23
    (1024, CBLK + 1024, 512, 0),     # blocks 24-27
    (1536, CBLK + 1536, 512, 0),     # blocks 28-31
    (2048, 2 * CBLK, 512, 1),        # blocks 32-35   shift-2
    (2560, 2 * CBLK + 512, 384, 1),  # blocks 36-38
]
# exp-sem count each wb waits for (from the highest block it reads);
# 0 = compact prefix only, no expansion dependency.
# left chunks drain on two engines: ACT owns chunks 0,2,4 (exp_la),
# DVE owns chunks 1,3,5 (exp_lb); per-wb (la, lb) minimums from the
# highest chunk needed: (0,1)->none; (1,6)->c2: (1,1); (7,7)->c4: (2,2);
# (14,7)->c5: (3,2); (21,3)->c6: (3,3)
_EXP_NEED_L = [(0, 0), (1, 1), (2, 2), (3, 2), (3, 3)]
# right chunks drain on two engines: DVE owns bank0/chunks 0,2,4 (exp_ra),
# ACT owns bank1/chunks 1,3,5 (exp_rb); per-wb (ra, rb) minimums:
_EXP_NEED_R = [(1, 1), (2, 2), (3, 2), (3, 3)]


def _build_nc():
    """One SPMD program for every core: 3 HWDGE loads (weights, compact
    left, full right), PE shift-matmul expansion of the left tile, a
    memset'd zero idx tile, 8 kv_writebacks (4 per half)."""
    nc = bacc.Bacc()
    wsh_d = nc.dram_tensor("wsh_d", [128, 256], _F32R, kind="ExternalInput")
    tl_d = nc.dram_tensor("tl_d", [128, CBLK], _F32R, kind="ExternalInput")
    tr_d = nc.dram_tensor("tr_d", [128, CBLK], _F32R, kind="ExternalInput")
    yl = nc.dram_tensor("yl", [LCORE * LVL + NPAD], _F32R, kind="ExternalOutput")
    yr = nc.dram_tensor("yr", [LCORE * LVL + NPAD], _F32R, kind="ExternalOutput")

    with (
        nc.Block() as block,
        nc.sbuf_tensor("tl", [128, FREE], _F32R) as tl,
        nc.sbuf_tensor("tr", [128, FREE], _F32R) as tr,
        nc.sbuf_tensor("wt", [128, 256], _F32R) as wt,
        nc.sbuf_tensor("ix", [128, 8], _I32) as ix,
        nc.psum_tensor("ps", [128, 3072], _F32) as ps,
        nc.psum_tensor("ps2", [128, 1024], _F32) as ps2,
        nc.semaphore("ld_w") as ld_w,
        nc.semaphore("ld_l") as ld_l,
        nc.semaphore("ld_r") as ld_r,
        nc.semaphore("mm") as mm,
        nc.semaphore("mm_r") as mm_r,
        nc.semaphore("exp_la") as exp_la,
        nc.semaphore("exp_lb") as exp_lb,
        nc.semaphore("exp_ra") as exp_ra,
        nc.semaphore("exp_rb") as exp_rb,
        nc.semaphore("ixs") as ixs,
        nc.semaphore("wbd") as wbd,
    ):
        # One ring, in order: weights (tiny) -> compact left -> full right,
        # so the left expansion chain starts as early as possible.
        @block.sync
        def _(sync):
            sync.dma_start(tl[:, :CBLK], tl_d[:, :]).then_inc(ld_l, 16)
            sync.dma_start(tr[:, :CBLK], tr_d[:, :]).then_inc(ld_r, 16)

        @block.scalar
        def _(scalar):
            scalar.dma_start(wt[:, :], wsh_d[:, :]).then_inc(ld_w, 16)

        @block.vector
        def _(vector):
            vector.memset(ix[:, :], 0).then_inc(ixs, 1)
            for k in (1, 3, 5):
                pcol, scol, n, _w = _CHUNKS[k]
                vector.wait_ge(mm, k + 1)
                vector.tensor_copy(
                    tl[:, scol : scol + n], ps[:, pcol : pcol + n]
                ).then_inc(exp_lb, 1)
            for c in (0, 2, 4):
                _pcol, scol, n, _w = _CHUNKS[c]
                vector.wait_ge(mm_r, c + 1)
                vector.tensor_copy(
                    tr[:, scol : scol + n], ps2[:, :n]
                ).then_inc(exp_ra, 1)

        @block.tensor
        def _(tensor):
            tensor.wait_ge(ld_w, 16)
            # warm the PE out of pstate-low before the latency-critical chain
            tensor.matmul(
                ps2[:, :128], wt[:, :128], wt[:, :128], start=True, stop=True
            )
            tensor.wait_ge(ld_l, 16)
            for pcol, _scol, n, w in _CHUNKS:
                # shift-1 chunks read source blocks [pcol/128, ...); shift-2
                # chunks (psum cols 2048+) read source blocks [(pcol-2048)/128, ...)
                src0 = pcol - 2048 if w else pcol
                tensor.matmul(
                    ps[:, pcol : pcol + n],
                    wt[:, 128 * w : 128 * (w + 1)],
                    tl[:, src0 : src0 + n],
                    start=True,
                    stop=True,
                ).then_inc(mm, 1)
            # right half: rotate through ps2's two banks; each bank is
            # drained by its own engine (DVE bank0, ACT bank1)
            tensor.wait_ge(ld_r, 16)
            for c, (pcol, _scol, n, w) in enumerate(_CHUNKS):
                if c >= 2:
                    tensor.wait_ge(exp_ra if c % 2 == 0 else exp_rb, c // 2)
                src0 = pcol - 2048 if w else pcol
                tensor.matmul(
                    ps2[:, (c % 2) * 512 : (c % 2) * 512 + n],
                    wt[:, 128 * w : 128 * (w + 1)],
                    tr[:, src0 : src0 + n],
                    start=True,
                    stop=True,
                ).then_inc(mm_r, 1)

        @block.scalar
        def _(scalar):
            for k in (0, 2, 4):
                pcol, scol, n, _w = _CHUNKS[k]
                scalar.wait_ge(mm, k + 1)
                scalar.copy(
                    tl[:, scol : scol + n], ps[:, pcol : pcol + n]
                ).then_inc(exp_la, 1)
            for c in (1, 3, 5):
                _pcol, scol, n, _w = _CHUNKS[c]
                scalar.wait_ge(mm_r, c + 1)
                scalar.copy(
                    tr[:, scol : scol + n], ps2[:, 512 : 512 + n]
                ).then_inc(exp_rb, 1)

        @block.gpsimd
        def _(gp):
            def wbs(y, t, lvl_stride, insts, exp_need):
                for i, (l0, L) in enumerate(insts):
                    la, lb = exp_need[i]
                    if la:
                        gp.wait_ge(exp_la, la)
                    if lb:
                        gp.wait_ge(exp_lb, lb)
                    out_ap = bass.AP(
                        y,
                        l0 * lvl_stride,
                        [[lvl_stride, L], [DHO * NCN, 128], [NCN, DHO], [1, NCTX]],
                    )
                    in_ap = bass.AP(
                        t, l0 * NCN, [[FREE, 128], [NCN, DHO], [NCN, L], [1, NCN]]
                    )
                    gp.kv_writeback(out_ap, in_ap, ix[:, :L]).then_inc(wbd, 16)

            gp.wait_ge(ixs, 1)
            gp.wait_ge(ld_l, 16)
            wbs(yl, tl, LVL + NCN, INSTS_L, _EXP_NEED_L)
            gp.wait_ge(ld_r, 16)
            for i, (l0, L) in enumerate(INSTS_R):
                ra, rb = _EXP_NEED_R[i]
                gp.wait_ge(exp_ra, ra)
                gp.wait_ge(exp_rb, rb)
                out_ap = bass.AP(
                    yr, l0 * LVL,
                    [[LVL, L], [DHO * NCN, 128], [NCN, DHO], [1, NCTX]],
                )
                in_ap = bass.AP(
                    tr, l0 * NCN, [[FREE, 128], [NCN, DHO], [NCN, L], [1, NCN]]
                )
                gp.kv_writeback(out_ap, in_ap, ix[:, :L]).then_inc(wbd, 16)
            gp.wait_ge(wbd, 16 * 9)

    nc.compile()
    return nc


def _get_nc():
    if "nc" not in _NC_CACHE:
        _NC_CACHE["nc"] = _build_nc()
    return _NC_CACHE["nc"]


# Partition content gather indices, precomputed once:
#   left : block w-index  M[s, t] = s*16 + t           (invalid -> zero)
#   right: block w-index  M[s, t] = 127 - s*16 - t     (invalid -> zero)
# right compact: partition (g,s) block u holds reversed content
# cr[g, 127 - (s*16 + u)]; all indices valid for u < 16.
_MR16 = 127 - ((np.arange(S) * DHO)[:, None] + np.arange(16)[None, :])


def _shift_weights():
    """W[k, m] = 1 selects source partition k = m + shift within each
    8-partition s-group (zero columns elsewhere)."""
    w = np.zeros((128, 256), np.float32)
    for m in range(128):
        if m % 8 < 7:
            w[m + 1, m] = 1.0          # W1: shift by one s-slot
        if m % 8 < 6:
            w[m + 2, 128 + m] = 1.0    # W2: shift by two s-slots
    return w


_WSH = _shift_weights()


def _run(left, right, **spmd_kwargs):
    from concourse.bass_utils import run_bass_kernel_spmd

    left = np.ascontiguousarray(np.asarray(left), dtype=np.float32)
    right = np.ascontiguousarray(np.asarray(right), dtype=np.float32)

    in_maps = []
    for k in range(NCORES):
        b, q = divmod(k, 2)
        # [g, w, cg] frames; ch = c*64 + h = g*128 + cg
        lw = left[b].reshape(G, 128, W).transpose(0, 2, 1)
        rw = right[b].reshape(G, 128, W).transpose(0, 2, 1)
        cl = np.zeros((G, 128, 128), np.float32)
        cl[:, : W - 24 * q, :] = lw[:, 24 * q :, :]          # bake level base
        # compact tiles: each partition (g,s) holds only its own 16 blocks
        tl_c = np.ascontiguousarray(cl.reshape(128, CBLK))
        tr_c = np.ascontiguousarray(rw[:, _MR16, :].reshape(128, CBLK))
        in_maps.append({"wsh_d": _WSH, "tl_d": tl_c, "tr_d": tr_c})

    res = run_bass_kernel_spmd(
        _get_nc(), in_maps, core_ids=list(range(NCORES)), **spmd_kwargs
    )

    out = np.zeros((B, 2 * C, D, H, W), np.float32)
    for k in range(NCORES):
        b, q = divmod(k, 2)
        wlim = W - 24 * q
        dsl = slice(24 * q, 24 * q + LCORE)
        wsl = slice(24 * q, W)

        def stitch(flat, rev):
            y = flat[: LCORE * LVL].reshape(LCORE, G, 128, 128)
            y = y[:, :, 128 - wlim :, :][:, :, ::-1, :] if rev else y[:, :, :wlim, :]
            # [dd, g, wl, cg] -> [c, dd, h, wl]
            y = y.transpose(1, 3, 0, 2).reshape(C, H, LCORE, wlim)
            return y.transpose(0, 2, 1, 3)

        out[b, 0:C, dsl, :, wsl] = stitch(res.results[k]["yl"], False)
        out[b, C:, dsl, :, wsl] = stitch(res.results[k]["yr"], True)
    return out, res


def kernel(left, right):
    out, _ = _run(left, right)
    return out


# revision 28
# speedup vs baseline: 1.2896x; 1.0235x over previous
"""Cost-volume concat kernel for Trainium2 (8 NeuronCores, SPMD).

Problem: left/right (B=4, C=32, H=64, W=128) f32 ->
         out (B, 2C, D=48, H, W) where
  out[b, c,    d, h, w] = left [b, c, h, w]     * (w >= d)
  out[b, C+c,  d, h, w] = right[b, c, h, w - d] * (w >= d)

Sharding: 8 cores = 4 batches x 2 level-halves (d in [24q, 24q+24)); every
core builds both the left and right channel halves for its 24 levels.

The stores go through gpsimd kv_writeback instead of plain DMA.  Work in a
channel-innermost frame: per core and half, y[dd, g, w, cg] with the 2048
(c, h) pairs split into g in [0,16) groups of cg in [0,128).  kv_writeback
writes, per batch entry b and partition p, dho runs of ncn contiguous
elements at per-batch column offset ctx_idx[b]:

  dst[b, p, j, idx[b] : idx[b]+ncn] , src[p, (j*batch_step + b)*ncn + k]

Mapping (ncn = 128, dho = 16, batch = 24 levels, partition p = g*8 + s):
  LEFT : chunk n = s*16 + j + dd at idx[dd] = dd*128; src block t = j + dd;
         partition (g,s) holds blocks P[t] = content[g, w = s*16 + t]
         (zero past w=127).  Writes n < dd are skipped => the masked
         prefix stays at the runtime's pre-zeroed fill; writes n >= 128
         overflow into the next row-group's zero prefix with zero data.
  RIGHT: stored w-REVERSED (host un-reverses): chunk n = s*16 + j at
         idx = 0; P[t] = content[g, w = 127 - s*16 - t] (zero for
         negative index), which makes the per-level source shift t = j+dd
         land on level-independent partition content, and the reversed
         mask zeros come from the same padding.

Level-base 24q is baked host-side: the left content is pre-shifted by 24q
columns, the right content is unshifted; all cores run an identical SPMD
program.  The left half's per-level dst shift idx[dd] = dd*128 is linear
in the batch index, so it is absorbed into batch_stride_bytes (LVL+128
elems per level) and the ctx_idxs are all-zero for both halves -- the idx
tile is a DVE memset, not a load.

Why kv_writeback: each instruction moves batch*2048 512B-runs but the DGE
costs descriptors per 16-partition stripe, so the store phase costs ~16x
less DMA-engine time than HWDGE dma_start (~9us instead of ~114us for the
two halves) and the kernel becomes load-bound.  Each half is 4 writebacks
of <=7 levels: one instruction's ~900 ring descriptors must fit the
default 1024-entry SWDGE carveout (the ucode ring size is fixed;
enlarging dynamic_dma_scratch_size crashes on HW).

The left tile's 39-block span repeats content across partitions
(partition p's blocks [16, 39) are partitions p+1 / p+2's blocks), so
only the unique 16-block prefix is DMA-loaded; the other 23 blocks are
produced on-chip by the otherwise-idle PE as partition-shift matmuls
(f32r, exact x*1+0 copies) through PSUM, drained to SBUF by the ACT
engine.  That takes 1.4MB off the serialized DMA device, and the left
desc-gens start as soon as the chunks they read are expanded.  The right
tile is loaded in full: its expansion would not shorten the critical
path (the Pool desc-gen chain overlaps the right load already).
"""

import sys
from contextlib import ExitStack

for _p in ("/opt/trn_rl_repo",):
    if _p not in sys.path:
        sys.path.append(_p)

import numpy as np

import concourse.bacc as bacc
import concourse.bass as bass
import concourse.mybir as mybir

B, C, H, W = 4, 32, 64, 128
D = 48
NCORES = 8
G, S, DHO, NCN = 16, 8, 16, 128
LCORE = D // 2                 # 24 levels per core
SPAN = DHO + LCORE - 1         # 39 blocks of 128 elems per partition
FREE = SPAN * NCN              # 4992 f32 per partition
LVL = G * 128 * NCN            # 262144 elems per level
NPAD = 8192                    # dram overflow pad (elems)
NCTX = 4096

_F32 = mybir.dt.float32
_F32R = mybir.dt.float32r
_I32 = mybir.dt.int32

_NC_CACHE = {}

# (level base, batch) per writeback; the left split leads with a
# compact-only instruction so its desc-gen starts before any expansion.
INSTS_L = [(0, 1), (1, 4), (5, 7), (12, 7), (19, 5)]
INSTS_R = [(0, 7), (7, 7), (14, 7), (21, 3)]
CBLK = 16 * NCN                # compact (DMA-loaded) prefix: 16 blocks
# expansion chunks: (psum col, sbuf col, ncols, W index)
_CHUNKS = [
    (0, CBLK, 512, 0),               # blocks 16-19   shift-1
    (512, CBLK + 512, 512, 0),       # blocks 20-23
    (1024, CBLK + 1024, 512, 0),     # blocks 24-27
    (1536, CBLK + 1536, 512, 0),     # blocks 28-31
    (2048, 2 * CBLK, 512, 1),        # blocks 32-35   shift-2
    (2560, 2 * CBLK + 512, 384, 1),  # blocks 36-38
]
# exp-sem count each wb waits for (from the highest block it reads);
# 0 = compact prefix only, no expansion dependency.
# left chunks drain on two engines: ACT owns chunks 0,2,4 (exp_la),
# DVE owns chunks 1,3,5 (exp_lb); per-wb (la, lb) minimums from the
# highest chunk needed: (0,1)->none; (1,6)->c2: (1,1); (7,7)->c4: (2,2);
# (14,7)->c5: (3,2); (21,3)->c6: (3,3)
_EXP_NEED_L = [(0, 0), (1, 0), (2, 1), (3, 2), (3, 3)]
# right chunks drain on two engines: DVE owns bank0/chunks 0,2,4 (exp_ra),
# ACT owns bank1/chunks 1,3,5 (exp_rb); per-wb (ra, rb) minimums:
_EXP_NEED_R = [(1, 1), (2, 2), (3, 2), (3, 3)]


def _build_nc():
    """One SPMD program for every core: 3 HWDGE loads (weights, compact
    left, full right), PE shift-matmul expansion of the left tile, a
    memset'd zero idx tile, 8 kv_writebacks (4 per half)."""
    nc = bacc.Bacc()
    wsh_d = nc.dram_tensor("wsh_d", [128, 256], _F32R, kind="ExternalInput")
    tl_d = nc.dram_tensor("tl_d", [128, CBLK], _F32R, kind="ExternalInput")
    tr_d = nc.dram_tensor("tr_d", [128, CBLK], _F32R, kind="ExternalInput")
    yl = nc.dram_tensor("yl", [LCORE * LVL + NPAD], _F32R, kind="ExternalOutput")
    yr = nc.dram_tensor("yr", [LCORE * LVL + NPAD], _F32R, kind="ExternalOutput")

    with (
        nc.Block() as block,
        nc.sbuf_tensor("tl", [128, FREE], _F32R) as tl,
        nc.sbuf_tensor("tr", [128, FREE], _F32R) as tr,
        nc.sbuf_tensor("wt", [128, 256], _F32R) as wt,
        nc.sbuf_tensor("ix", [128, 8], _I32) as ix,
        nc.psum_tensor("ps", [128, 3072], _F32) as ps,
        nc.psum_tensor("ps2", [128, 1024], _F32) as ps2,
        nc.semaphore("ld_w") as ld_w,
        nc.semaphore("ld_l") as ld_l,
        nc.semaphore("ld_r") as ld_r,
        nc.semaphore("mm") as mm,
        nc.semaphore("mm_r") as mm_r,
        nc.semaphore("exp_la") as exp_la,
        nc.semaphore("exp_lb") as exp_lb,
        nc.semaphore("exp_ra") as exp_ra,
        nc.semaphore("exp_rb") as exp_rb,
        nc.semaphore("ixs") as ixs,
        nc.semaphore("wbd") as wbd,
    ):
        # One ring, in order: weights (tiny) -> compact left -> full right,
        # so the left expansion chain starts as early as possible.
        @block.sync
        def _(sync):
            sync.dma_start(tl[:, :CBLK], tl_d[:, :]).then_inc(ld_l, 16)
            sync.dma_start(tr[:, :CBLK], tr_d[:, :]).then_inc(ld_r, 16)

        @block.scalar
        def _(scalar):
            scalar.dma_start(wt[:, :], wsh_d[:, :]).then_inc(ld_w, 16)

        @block.vector
        def _(vector):
            vector.memset(ix[:, :], 0).then_inc(ixs, 1)
            for k in (1, 3, 5):
                pcol, scol, n, _w = _CHUNKS[k]
                vector.wait_ge(mm, k + 1)
                vector.tensor_copy(
                    tl[:, scol : scol + n], ps[:, pcol : pcol + n]
                ).then_inc(exp_lb, 1)
            for c in (0, 2, 4):
                _pcol, scol, n, _w = _CHUNKS[c]
                vector.wait_ge(mm_r, c + 1)
                vector.tensor_copy(
                    tr[:, scol : scol + n], ps2[:, :n]
                ).then_inc(exp_ra, 1)

        @block.tensor
        def _(tensor):
            tensor.wait_ge(ld_w, 16)
            # warm the PE out of pstate-low before the latency-critical chain
            tensor.matmul(
                ps2[:, :128], wt[:, :128], wt[:, :128], start=True, stop=True
            )
            tensor.wait_ge(ld_l, 16)
            for pcol, _scol, n, w in _CHUNKS:
                # shift-1 chunks read source blocks [pcol/128, ...); shift-2
                # chunks (psum cols 2048+) read source blocks [(pcol-2048)/128, ...)
                src0 = pcol - 2048 if w else pcol
                tensor.matmul(
                    ps[:, pcol : pcol + n],
                    wt[:, 128 * w : 128 * (w + 1)],
                    tl[:, src0 : src0 + n],
                    start=True,
                    stop=True,
                ).then_inc(mm, 1)
            # right half: rotate through ps2's two banks; each bank is
            # drained by its own engine (DVE bank0, ACT bank1)
            tensor.wait_ge(ld_r, 16)
            for c, (pcol, _scol, n, w) in enumerate(_CHUNKS):
                if c >= 2:
                    tensor.wait_ge(exp_ra if c % 2 == 0 else exp_rb, c // 2)
                src0 = pcol - 2048 if w else pcol
                tensor.matmul(
                    ps2[:, (c % 2) * 512 : (c % 2) * 512 + n],
                    wt[:, 128 * w : 128 * (w + 1)],
                    tr[:, src0 : src0 + n],
                    start=True,
                    stop=True,
                ).then_inc(mm_r, 1)

        @block.scalar
        def _(scalar):
            for k in (0, 2, 4):
                pcol, scol, n, _w = _CHUNKS[k]
                scalar.wait_ge(mm, k + 1)
                scalar.copy(
                    tl[:, scol : scol + n], ps[:, pcol : pcol + n]
                ).then_inc(exp_la, 1)
            for c in (1, 3, 5):
                _pcol, scol, n, _w = _CHUNKS[c]
                scalar.wait_ge(mm_r, c + 1)
                scalar.copy(
                    tr[:, scol : scol + n], ps2[:, 512 : 512 + n]
                ).then_inc(exp_rb, 1)

        @block.gpsimd
        def _(gp):
            def wbs(y, t, lvl_stride, insts, exp_need):
                for i, (l0, L) in enumerate(insts):
                    la, lb = exp_need[i]
                    if la:
                        gp.wait_ge(exp_la, la)
                    if lb:
                        gp.wait_ge(exp_lb, lb)
                    out_ap = bass.AP(
                        y,
                        l0 * lvl_stride,
                        [[lvl_stride, L], [DHO * NCN, 128], [NCN, DHO], [1, NCTX]],
                    )
                    in_ap = bass.AP(
                        t, l0 * NCN, [[FREE, 128], [NCN, DHO], [NCN, L], [1, NCN]]
                    )
                    gp.kv_writeback(out_ap, in_ap, ix[:, :L]).then_inc(wbd, 16)

            gp.wait_ge(ixs, 1)
            gp.wait_ge(ld_l, 16)
            wbs(yl, tl, LVL + NCN, INSTS_L, _EXP_NEED_L)
            gp.wait_ge(ld_r, 16)
            for i, (l0, L) in enumerate(INSTS_R):
                ra, rb = _EXP_NEED_R[i]
                gp.wait_ge(exp_ra, ra)
                gp.wait_ge(exp_rb, rb)
                out_ap = bass.AP(
                    yr, l0 * LVL,
                    [[LVL, L], [DHO * NCN, 128], [NCN, DHO], [1, NCTX]],
                )
                in_ap = bass.AP(
                    tr, l0 * NCN, [[FREE, 128], [NCN, DHO], [NCN, L], [1, NCN]]
                )
                gp.kv_writeback(out_ap, in_ap, ix[:, :L]).then_inc(wbd, 16)
            gp.wait_ge(wbd, 16 * 9)

    nc.compile()
    return nc


def _get_nc():
    if "nc" not in _NC_CACHE:
        _NC_CACHE["nc"] = _build_nc()
    return _NC_CACHE["nc"]


# Partition content gather indices, precomputed once:
#   left : block w-index  M[s, t] = s*16 + t           (invalid -> zero)
#   right: block w-index  M[s, t] = 127 - s*16 - t     (invalid -> zero)
# right compact: partition (g,s) block u holds reversed content
# cr[g, 127 - (s*16 + u)]; all indices valid for u < 16.
_MR16 = 127 - ((np.arange(S) * DHO)[:, None] + np.arange(16)[None, :])


def _shift_weights():
    """W[k, m] = 1 selects source partition k = m + shift within each
    8-partition s-group (zero columns elsewhere)."""
    w = np.zeros((128, 256), np.float32)
    for m in range(128):
        if m % 8 < 7:
            w[m + 1, m] = 1.0          # W1: shift by one s-slot
        if m % 8 < 6:
            w[m + 2, 128 + m] = 1.0    # W2: shift by two s-slots
    return w


_WSH = _shift_weights()


def _run(left, right, **spmd_kwargs):
    from concourse.bass_utils import run_bass_kernel_spmd

    left = np.ascontiguousarray(np.asarray(left), dtype=np.float32)
    right = np.ascontiguousarray(np.asarray(right), dtype=np.float32)

    in_maps = []
    for k in range(NCORES):
        b, q = divmod(k, 2)
        # [g, w, cg] frames; ch = c*64 + h = g*128 + cg
        lw = left[b].reshape(G, 128, W).transpose(0, 2, 1)
        rw = right[b].reshape(G, 128, W).transpose(0, 2, 1)
        cl = np.zeros((G, 128, 128), np.float32)
        cl[:, : W - 24 * q, :] = lw[:, 24 * q :, :]          # bake level base
        # compact tiles: each partition (g,s) holds only its own 16 blocks
        tl_c = np.ascontiguousarray(cl.reshape(128, CBLK))
        tr_c = np.ascontiguousarray(rw[:, _MR16, :].reshape(128, CBLK))
        in_maps.append({"wsh_d": _WSH, "tl_d": tl_c, "tr_d": tr_c})

    res = run_bass_kernel_spmd(
        _get_nc(), in_maps, core_ids=list(range(NCORES)), **spmd_kwargs
    )

    out = np.zeros((B, 2 * C, D, H, W), np.float32)
    for k in range(NCORES):
        b, q = divmod(k, 2)
        wlim = W - 24 * q
        dsl = slice(24 * q, 24 * q + LCORE)
        wsl = slice(24 * q, W)

        def stitch(flat, rev):
            y = flat[: LCORE * LVL].reshape(LCORE, G, 128, 128)
            y = y[:, :, 128 - wlim :, :][:, :, ::-1, :] if rev else y[:, :, :wlim, :]
            # [dd, g, wl, cg] -> [c, dd, h, wl]
            y = y.transpose(1, 3, 0, 2).reshape(C, H, LCORE, wlim)
            return y.transpose(0, 2, 1, 3)

        out[b, 0:C, dsl, :, wsl] = stitch(res.results[k]["yl"], False)
        out[b, C:, dsl, :, wsl] = stitch(res.results[k]["yr"], True)
    return out, res


def kernel(left, right):
    out, _ = _run(left, right)
    return out
